# revision 1
# baseline (speedup 1.0000x reference)
"""MoE (16 routed experts, top-2, + shared expert) on 8 TRN2 NeuronCores.

Strategy (expert-parallel per the sharding hint):
  Launch A (SPMD, data-parallel over tokens): each core takes a 2048-token
    slice, computes router logits/softmax/top-2 combine weights on-device
    (fp32 matmul for exact-ish selection) and the shared-expert SwiGLU FFN
    (fp32r matmuls = bf16-speed).  Outputs: comb (2048x16), shared y^T.
  Host: reads comb, builds per-expert token index lists, gathers token
    vectors into dense per-expert batches (the "all-to-all dispatch").
  Launch B (SPMD, expert-parallel): core c owns experts 2c and 2c+1; runs
    the SwiGLU FFN on each expert's gathered batch, scaling rows by the
    combine weight on-device.  Outputs: weighted y^T per expert.
  Host: scatter-adds expert outputs + shared outputs into the full result
    (the "combine").

All activations travel transposed (feature-major, token-minor) so every
matmul operand loads with natural DMA strides and zero on-device transposes.
"""

import math

import numpy as np

# model dims (fixed for this problem)
E, TOPK, C, I = 16, 2, 768, 1536
B, T = 8, 2048
NCORE = 8
NTOK = B * T           # 16384
TPC = NTOK // NCORE    # 2048 tokens per core
CK = C // 128          # 6 contraction chunks for C
IK = I // 128          # 12 chunks for I
NBLK = 512             # token block = PE moving-dim per matmul

TRACE = False          # set True (from a driver) to capture NTFF timing
LAST = {}              # timing info from the most recent kernel() call

_progs = {}            # compiled program cache


def _enable_axon_ntff_profiling():
    import sys
    import types

    if "antenv.axon_hooks" not in sys.modules:
        mod = types.ModuleType("antenv.axon_hooks")
        mod._hook = None
        mod.set_axon_ntff_profile_hook = lambda h: setattr(mod, "_hook", h)
        mod.get_axon_ntff_profile_hook = lambda: mod._hook
        sys.modules["antenv.axon_hooks"] = mod
    from antenv.axon_hooks import set_axon_ntff_profile_hook  # type: ignore
    from trn_agent_boot.trn_boot import _ntff_profile_via_ctypes

    set_axon_ntff_profile_hook(_ntff_profile_via_ctypes("/opt/axon/libaxon_pjrt.so"))
    import concourse.bass_utils as bu

    bu.upload_artifacts = lambda tmpdir: f"file://{tmpdir}"


def _blocks(m):
    """Split m tokens into PE-friendly blocks (<=512 each)."""
    out = []
    n0 = 0
    while n0 < m:
        nb = min(NBLK, m - n0)
        out.append((n0, nb))
        n0 += nb
    return out


def _emit_ffn_block(nc, pools, x_all, wg_sb, wu_sb, wd_sb, scale_sb, y_ap, n0, nblk):
    """One token-block of SwiGLU FFN in transposed layout.

    x_all: SBUF [128, CK, NBLK] (c-major, token-minor) for this block
    wg_sb/wu_sb: SBUF [128, CK, I]; wd_sb: SBUF [128, IK, C]
    scale_sb: SBUF [128, cap] per-token combine weight (or None)
    y_ap: DRAM (C, M) output, written at columns [n0, n0+nblk)
    """
    import concourse.mybir as mybir

    f32 = mybir.dt.float32
    f32r = mybir.dt.float32r
    hpool, gpool, ypool, pgu, pd = (
        pools["h"],
        pools["g"],
        pools["y"],
        pools["pgu"],
        pools["pd"],
    )

    h_all = hpool.tile([128, IK, NBLK], f32r, tag="h_all")
    for ik in range(IK):
        psg = pgu.tile([128, NBLK], f32, tag="psg")
        psu = pgu.tile([128, NBLK], f32, tag="psu")
        for ck in range(CK):
            nc.tensor.matmul(
                psg[:, :nblk],
                lhsT=wg_sb[:, ck, ik * 128 : (ik + 1) * 128],
                rhs=x_all[:, ck, :nblk],
                start=(ck == 0),
                stop=(ck == CK - 1),
            )
        for ck in range(CK):
            nc.tensor.matmul(
                psu[:, :nblk],
                lhsT=wu_sb[:, ck, ik * 128 : (ik + 1) * 128],
                rhs=x_all[:, ck, :nblk],
                start=(ck == 0),
                stop=(ck == CK - 1),
            )
        ga = gpool.tile([128, NBLK], f32, tag="ga")
        nc.scalar.activation(
            ga[:, :nblk], psg[:, :nblk], mybir.ActivationFunctionType.Silu
        )
        nc.vector.tensor_mul(h_all[:, ik, :nblk], ga[:, :nblk], psu[:, :nblk])

    for ck in range(CK):
        psd = pd.tile([128, NBLK], f32, tag="psd")
        for ik in range(IK):
            nc.tensor.matmul(
                psd[:, :nblk],
                lhsT=wd_sb[:, ik, ck * 128 : (ck + 1) * 128],
                rhs=h_all[:, ik, :nblk],
                start=(ik == 0),
                stop=(ik == IK - 1),
            )
        yb = ypool.tile([128, NBLK], f32, tag="yb")
        if scale_sb is None:
            nc.vector.tensor_copy(yb[:, :nblk], psd[:, :nblk])
        else:
            nc.vector.tensor_mul(
                yb[:, :nblk], psd[:, :nblk], scale_sb[:, n0 : n0 + nblk]
            )
        nc.sync.dma_start(
            out=y_ap[ck * 128 : (ck + 1) * 128, n0 : n0 + nblk], in_=yb[:, :nblk]
        )


def _build_launch_a():
    """Router + shared expert, one 2048-token slice per core."""
    from contextlib import ExitStack

    import concourse.tile as tile
    from concourse import bacc, mybir

    f32 = mybir.dt.float32
    AX = mybir.AxisListType.X
    OP = mybir.AluOpType

    nc = bacc.Bacc("TRN2", target_bir_lowering=False, debug=False)
    f32r = mybir.dt.float32r
    xt_ap = nc.dram_tensor("xt", [C, TPC], f32, kind="ExternalInput").ap()
    wgate_ap = nc.dram_tensor("wgate", [C, E], f32, kind="ExternalInput").ap()
    biasb_ap = nc.dram_tensor("biasb", [128, E], f32, kind="ExternalInput").ap()
    swg_ap = nc.dram_tensor("swg", [C, I], f32r, kind="ExternalInput").ap()
    swu_ap = nc.dram_tensor("swu", [C, I], f32r, kind="ExternalInput").ap()
    swd_ap = nc.dram_tensor("swd", [I, C], f32r, kind="ExternalInput").ap()
    comb_ap = nc.dram_tensor("comb", [TPC, E], f32, kind="ExternalOutput").ap()
    yst_ap = nc.dram_tensor("yst", [C, TPC], f32, kind="ExternalOutput").ap()

    with tile.TileContext(nc) as tc, ExitStack() as ctx:
        wpool = ctx.enter_context(tc.tile_pool(name="weights", bufs=1))
        xpool = ctx.enter_context(tc.tile_pool(name="xp", bufs=2))
        hpool = ctx.enter_context(tc.tile_pool(name="hp", bufs=1))
        gpool = ctx.enter_context(tc.tile_pool(name="gp", bufs=2))
        ypool = ctx.enter_context(tc.tile_pool(name="yp", bufs=3))
        rpool = ctx.enter_context(tc.tile_pool(name="rp", bufs=2))
        pgu = ctx.enter_context(tc.tile_pool(name="pgu", bufs=2, space="PSUM"))
        pd = ctx.enter_context(tc.tile_pool(name="pd", bufs=2, space="PSUM"))
        pr = ctx.enter_context(tc.tile_pool(name="pr", bufs=2, space="PSUM"))
        xrpool = ctx.enter_context(tc.tile_pool(name="xr", bufs=1))
        pools = {"h": hpool, "g": gpool, "y": ypool, "pgu": pgu, "pd": pd}

        wgate_sb = wpool.tile([128, CK, E], f32, tag="wgate")
        swg_sb = wpool.tile([128, CK, I], f32r, tag="swg")
        swu_sb = wpool.tile([128, CK, I], f32r, tag="swu")
        swd_sb = wpool.tile([128, IK, C], f32r, tag="swd")
        bias_sb = wpool.tile([128, E], f32, tag="bias")
        for ck in range(CK):
            nc.sync.dma_start(
                out=swg_sb[:, ck, :], in_=swg_ap[ck * 128 : (ck + 1) * 128, :]
            )
        for ck in range(CK):
            nc.sync.dma_start(
                out=wgate_sb[:, ck, :], in_=wgate_ap[ck * 128 : (ck + 1) * 128, :]
            )
        nc.sync.dma_start(out=bias_sb[:], in_=biasb_ap[:])
        for ck in range(CK):
            nc.sync.dma_start(
                out=swu_sb[:, ck, :], in_=swu_ap[ck * 128 : (ck + 1) * 128, :]
            )
        for ik in range(IK):
            nc.sync.dma_start(
                out=swd_sb[:, ik, :], in_=swd_ap[ik * 128 : (ik + 1) * 128, :]
            )

        for n in range(TPC // NBLK):
            x32 = xpool.tile([128, CK, NBLK], f32, tag="x32")
            for ck in range(CK):
                nc.sync.dma_start(
                    out=x32[:, ck, :],
                    in_=xt_ap[ck * 128 : (ck + 1) * 128, n * NBLK : (n + 1) * NBLK],
                )
            x_all = xrpool.tile([128, CK, NBLK], f32r, tag="x_all")
            nc.vector.tensor_copy(x_all[:], x32[:])
            # router: tokens as PSUM partitions, 4 chunks of 128 per block
            for q in range(NBLK // 128):
                t0 = q * 128
                psl = pr.tile([128, E], f32, tag="psl")
                for ck in range(CK):
                    nc.tensor.matmul(
                        psl[:],
                        lhsT=x32[:, ck, t0 : t0 + 128],
                        rhs=wgate_sb[:, ck, :],
                        start=(ck == 0),
                        stop=(ck == CK - 1),
                    )
                lg = rpool.tile([128, E], f32, tag="lg")
                nc.vector.tensor_add(lg[:], psl[:], bias_sb[:])
                m1 = rpool.tile([128, 1], f32, tag="m1")
                nc.vector.reduce_max(m1[:], lg[:], axis=AX)
                nm1 = rpool.tile([128, 1], f32, tag="nm1")
                nc.vector.tensor_scalar_mul(nm1[:], m1[:], -1.0)
                ex = rpool.tile([128, E], f32, tag="ex")
                nc.scalar.activation(
                    ex[:], lg[:], mybir.ActivationFunctionType.Exp, bias=nm1[:]
                )
                msk1 = rpool.tile([128, E], f32, tag="msk1")
                nc.vector.tensor_scalar(msk1[:], lg[:], m1[:], None, op0=OP.is_equal)
                pen = rpool.tile([128, E], f32, tag="pen")
                nc.vector.tensor_scalar_mul(pen[:], msk1[:], 1e30)
                lm = rpool.tile([128, E], f32, tag="lm")
                nc.vector.tensor_sub(lm[:], lg[:], pen[:])
                m2 = rpool.tile([128, 1], f32, tag="m2")
                nc.vector.reduce_max(m2[:], lm[:], axis=AX)
                ge = rpool.tile([128, E], f32, tag="ge")
                nc.vector.tensor_scalar(ge[:], lg[:], m2[:], None, op0=OP.is_ge)
                we = rpool.tile([128, E], f32, tag="we")
                nc.vector.tensor_mul(we[:], ex[:], ge[:])
                sm = rpool.tile([128, 1], f32, tag="sm")
                nc.vector.reduce_sum(sm[:], we[:], axis=AX)
                rs = rpool.tile([128, 1], f32, tag="rs")
                nc.vector.reciprocal(rs[:], sm[:])
                cmb = rpool.tile([128, E], f32, tag="cmb")
                nc.vector.tensor_scalar(cmb[:], we[:], rs[:], None, op0=OP.mult)
                nc.sync.dma_start(
                    out=comb_ap[n * NBLK + t0 : n * NBLK + t0 + 128, :], in_=cmb[:]
                )
            # shared expert FFN on this block
            _emit_ffn_block(
                nc, pools, x_all, swg_sb, swu_sb, swd_sb, None, yst_ap, n * NBLK, NBLK
            )

    nc.compile()
    return nc


def _build_launch_b(cap):
    """Two routed experts per core on dense gathered batches of size cap."""
    from contextlib import ExitStack

    import concourse.tile as tile
    from concourse import bacc, mybir

    f32 = mybir.dt.float32
    f32r = mybir.dt.float32r

    nc = bacc.Bacc("TRN2", target_bir_lowering=False, debug=False)
    aps = {}
    for s in ("a", "b"):
        aps[f"x{s}"] = nc.dram_tensor(f"x{s}t", [C, cap], f32r, kind="ExternalInput").ap()
        aps[f"wg{s}"] = nc.dram_tensor(f"wg{s}", [C, I], f32r, kind="ExternalInput").ap()
        aps[f"wu{s}"] = nc.dram_tensor(f"wu{s}", [C, I], f32r, kind="ExternalInput").ap()
        aps[f"wd{s}"] = nc.dram_tensor(f"wd{s}", [I, C], f32r, kind="ExternalInput").ap()
        aps[f"sc{s}"] = nc.dram_tensor(f"sc{s}", [128, cap], f32, kind="ExternalInput").ap()
        aps[f"y{s}"] = nc.dram_tensor(f"y{s}t", [C, cap], f32, kind="ExternalOutput").ap()

    with tile.TileContext(nc) as tc, ExitStack() as ctx:
        wpool = ctx.enter_context(tc.tile_pool(name="weights", bufs=1))
        xpool = ctx.enter_context(tc.tile_pool(name="xp", bufs=2))
        hpool = ctx.enter_context(tc.tile_pool(name="hp", bufs=1))
        gpool = ctx.enter_context(tc.tile_pool(name="gp", bufs=2))
        ypool = ctx.enter_context(tc.tile_pool(name="yp", bufs=3))
        spool = ctx.enter_context(tc.tile_pool(name="sp", bufs=1))
        pgu = ctx.enter_context(tc.tile_pool(name="pgu", bufs=2, space="PSUM"))
        pd = ctx.enter_context(tc.tile_pool(name="pd", bufs=2, space="PSUM"))
        pools = {"h": hpool, "g": gpool, "y": ypool, "pgu": pgu, "pd": pd}

        for s in ("a", "b"):
            wg_sb = wpool.tile([128, CK, I], f32r, tag="wg")
            wu_sb = wpool.tile([128, CK, I], f32r, tag="wu")
            wd_sb = wpool.tile([128, IK, C], f32r, tag="wd")
            sc_sb = spool.tile([128, cap], f32, tag="sc")
            for ck in range(CK):
                nc.sync.dma_start(
                    out=wg_sb[:, ck, :], in_=aps[f"wg{s}"][ck * 128 : (ck + 1) * 128, :]
                )
            for ck in range(CK):
                nc.sync.dma_start(
                    out=wu_sb[:, ck, :], in_=aps[f"wu{s}"][ck * 128 : (ck + 1) * 128, :]
                )
            for ik in range(IK):
                nc.sync.dma_start(
                    out=wd_sb[:, ik, :], in_=aps[f"wd{s}"][ik * 128 : (ik + 1) * 128, :]
                )
            nc.sync.dma_start(out=sc_sb[:], in_=aps[f"sc{s}"][:])
            for n0, nblk in _blocks(cap):
                x_all = xpool.tile([128, CK, NBLK], f32r, tag="x_all")
                for ck in range(CK):
                    nc.sync.dma_start(
                        out=x_all[:, ck, :nblk],
                        in_=aps[f"x{s}"][ck * 128 : (ck + 1) * 128, n0 : n0 + nblk],
                    )
                _emit_ffn_block(
                    nc, pools, x_all, wg_sb, wu_sb, wd_sb, sc_sb, aps[f"y{s}"], n0, nblk
                )

    nc.compile()
    return nc


def _run(nc, in_maps, tag):
    from concourse.bass_utils import run_bass_kernel_spmd

    if TRACE:
        _enable_axon_ntff_profiling()
        res = run_bass_kernel_spmd(nc, in_maps, list(range(NCORE)), trace=True)
        LAST[f"{tag}_ns"] = res.exec_time_ns
        if res.instructions_and_trace is not None:
            LAST[f"{tag}_trace"] = res.instructions_and_trace[1]
    else:
        res = run_bass_kernel_spmd(nc, in_maps, list(range(NCORE)), trace=False)
    return res.results


def kernel(x, w_gate, expert_bias, wg, wu, wd, swg, swu, swd):
    LAST.clear()
    xf = np.ascontiguousarray(np.asarray(x, np.float32).reshape(NTOK, C))
    w_gate = np.ascontiguousarray(np.asarray(w_gate, np.float32))
    expert_bias = np.asarray(expert_bias, np.float32)
    wg = np.asarray(wg, np.float32)
    wu = np.asarray(wu, np.float32)
    wd = np.asarray(wd, np.float32)
    swg = np.ascontiguousarray(np.asarray(swg, np.float32))
    swu = np.ascontiguousarray(np.asarray(swu, np.float32))
    swd = np.ascontiguousarray(np.asarray(swd, np.float32))

    xt_full = np.ascontiguousarray(xf.T)  # (C, NTOK)
    bias_b = np.ascontiguousarray(np.broadcast_to(expert_bias, (128, E)))

    # ---- launch A: router + shared expert
    if "A" not in _progs:
        _progs["A"] = _build_launch_a()
    in_maps = []
    for c in range(NCORE):
        in_maps.append(
            {
                "xt": np.ascontiguousarray(xt_full[:, c * TPC : (c + 1) * TPC]),
                "wgate": w_gate,
                "biasb": bias_b,
                "swg": swg,
                "swu": swu,
                "swd": swd,
            }
        )
    res_a = _run(_progs["A"], in_maps, "launchA")

    comb = np.concatenate([res_a[c]["comb"] for c in range(NCORE)], axis=0)

    # ---- host routing: per-expert index lists + weights
    idxs, wts = [], []
    for e in range(E):
        ii = np.nonzero(comb[:, e] > 0.0)[0]
        idxs.append(ii)
        wts.append(comb[ii, e].astype(np.float32))
    max_cnt = max(len(ii) for ii in idxs)
    cap = max(NBLK, ((max_cnt + 127) // 128) * 128)

    # ---- launch B: routed experts (2 per core)
    key = ("B", cap)
    if key not in _progs:
        _progs[key] = _build_launch_b(cap)
    in_maps_b = []
    for c in range(NCORE):
        m = {}
        for s, e in (("a", 2 * c), ("b", 2 * c + 1)):
            ii, ww = idxs[e], wts[e]
            xt = np.zeros((C, cap), np.float32)
            xt[:, : len(ii)] = xf[ii].T
            sc = np.zeros((128, cap), np.float32)
            sc[:, : len(ii)] = ww[None, :]
            m[f"x{s}t"] = xt
            m[f"sc{s}"] = sc
            m[f"wg{s}"] = np.ascontiguousarray(wg[e])
            m[f"wu{s}"] = np.ascontiguousarray(wu[e])
            m[f"wd{s}"] = np.ascontiguousarray(wd[e])
        in_maps_b.append(m)
    res_b = _run(_progs[key], in_maps_b, "launchB")

    # ---- host combine: shared + scattered weighted expert outputs
    out = np.empty((NTOK, C), np.float32)
    for c in range(NCORE):
        out[c * TPC : (c + 1) * TPC] = res_a[c]["yst"].T
    for e in range(E):
        c, s = e // 2, ("a", "b")[e % 2]
        y = res_b[c][f"y{s}t"]  # (C, cap), already comb-weighted
        out[idxs[e]] += y[:, : len(idxs[e])].T

    if TRACE:
        LAST["total_ns"] = sum(
            v for k, v in LAST.items() if isinstance(v, int) and k.endswith("_ns")
        )
    return out.reshape(B, T, C)



# revision 2
# speedup vs baseline: 1.3215x; 1.3215x over previous
"""MoE (16 routed experts, top-2, + shared expert) on 8 TRN2 NeuronCores.

Strategy (expert-parallel per the sharding hint):
  Host: router (x @ w_gate + bias, softmax, top-2, renormalize) — 0.1% of
    total FLOPs — plus the all-to-all dispatch: per-expert token gather into
    dense padded batches.  Experts are pair-balanced across cores (sorted
    pairing: core i gets the i-th largest and i-th smallest expert batch) so
    per-core padded work is near-uniform and minimal.
  Device (ONE SPMD launch, all 8 cores): core c runs the shared-expert
    SwiGLU FFN on its 2048-token slice, then the FFNs of its two routed
    experts on their gathered batches.  All matmul data travels bf16
    (PE runs bf16 at the same rate as fp32r, but DMA bytes halve); PSUM
    accumulation is fp32 and outputs are written fp32.
  Host: combine — scatter-add comb-weighted expert outputs + shared output.

Layout: all activations travel transposed (feature-major, token-minor).
Gate/up weights are host-packed per-ik ([IK*128, CK*128] with (ck, icol)
free order) so each 128-wide I-chunk loads as one contiguous DMA and the
first matmuls can start ~2us into the launch.  Weight DMAs ride SWDGE
(gpsimd) so their tile-recycle waits never block the x/y HWDGE queue.
"""

import numpy as np

# model dims (fixed for this problem)
E, TOPK, C, I = 16, 2, 768, 1536
B, T = 8, 2048
NCORE = 8
NTOK = B * T           # 16384
TPC = NTOK // NCORE    # 2048 tokens per core
CK = C // 128          # 6 contraction chunks for C
IK = I // 128          # 12 chunks for I
NBLK = 512             # token block = PE moving-dim per matmul

TRACE = False          # set True (from a driver) to capture NTFF timing
LAST = {}              # timing info from the most recent kernel() call

_progs = {}            # compiled program cache


def _bf16():
    import ml_dtypes

    return ml_dtypes.bfloat16


def _enable_axon_ntff_profiling():
    import sys
    import types

    if "antenv.axon_hooks" not in sys.modules:
        mod = types.ModuleType("antenv.axon_hooks")
        mod._hook = None
        mod.set_axon_ntff_profile_hook = lambda h: setattr(mod, "_hook", h)
        mod.get_axon_ntff_profile_hook = lambda: mod._hook
        sys.modules["antenv.axon_hooks"] = mod
    from antenv.axon_hooks import set_axon_ntff_profile_hook  # type: ignore
    from trn_agent_boot.trn_boot import _ntff_profile_via_ctypes

    set_axon_ntff_profile_hook(_ntff_profile_via_ctypes("/opt/axon/libaxon_pjrt.so"))
    import concourse.bass_utils as bu

    bu.upload_artifacts = lambda tmpdir: f"file://{tmpdir}"


def _blocks(m):
    out = []
    n0 = 0
    while n0 < m:
        nb = min(NBLK, m - n0)
        out.append((n0, nb))
        n0 += nb
    return out


def _pack_gu(w):
    """[C, I] -> [IK*128, CK*128] bf16 so row-block ik is one contiguous
    [128, 768] DMA whose free order is (ck, icol)."""
    p = w.reshape(CK, 128, IK, 128).transpose(2, 1, 0, 3).reshape(IK * 128, CK * 128)
    return np.ascontiguousarray(p.astype(_bf16()))


def _emit_ffn_phase(nc, tc, pools, aps, tag, cap):
    """Full SwiGLU FFN phase: y[C, cap] = down(silu(gate(x)) * up(x)).

    aps: dict with x (DRAM [C, cap] bf16), y (DRAM [C, cap] f32),
    wg/wu (DRAM [IK*128, CK*128] bf16 packed), wd (DRAM [I, C] bf16).
    """
    import concourse.mybir as mybir

    f32 = mybir.dt.float32
    bf16 = mybir.dt.bfloat16
    wpool, xpool, hpool, gpool, ypool, pgu, pd = (
        pools["w"],
        pools["x"],
        pools["h"],
        pools["g"],
        pools["y"],
        pools["pgu"],
        pools["pd"],
    )

    # weight tiles: per-ik [128, CK*128] (gate/up) and [128, C] (down);
    # SWDGE so a blocked recycle-wait never stalls the x/y HWDGE queue.
    wg_t, wu_t, wd_t = [], [], []
    for ik in range(IK):
        g = wpool.tile([128, CK * 128], bf16, tag=f"wg{ik}")
        u = wpool.tile([128, CK * 128], bf16, tag=f"wu{ik}")
        nc.gpsimd.dma_start(out=g[:], in_=aps["wg"][ik * 128 : (ik + 1) * 128, :])
        nc.gpsimd.dma_start(out=u[:], in_=aps["wu"][ik * 128 : (ik + 1) * 128, :])
        wg_t.append(g)
        wu_t.append(u)
    for ik in range(IK):
        dt_ = wpool.tile([128, C], bf16, tag=f"wd{ik}")
        nc.gpsimd.dma_start(out=dt_[:], in_=aps["wd"][ik * 128 : (ik + 1) * 128, :])
        wd_t.append(dt_)

    for n0, nblk in _blocks(cap):
        x_t = xpool.tile([128, CK, NBLK], bf16, tag="x")
        for ck in range(CK):
            nc.sync.dma_start(
                out=x_t[:, ck, :nblk],
                in_=aps["x"][ck * 128 : (ck + 1) * 128, n0 : n0 + nblk],
            )
        h_t = hpool.tile([128, IK, NBLK], bf16, tag="h")
        for ik in range(IK):
            psg = pgu.tile([128, NBLK], f32, tag="psg")
            psu = pgu.tile([128, NBLK], f32, tag="psu")
            for ck in range(CK):
                nc.tensor.matmul(
                    psg[:, :nblk],
                    lhsT=wg_t[ik][:, ck * 128 : (ck + 1) * 128],
                    rhs=x_t[:, ck, :nblk],
                    start=(ck == 0),
                    stop=(ck == CK - 1),
                )
            for ck in range(CK):
                nc.tensor.matmul(
                    psu[:, :nblk],
                    lhsT=wu_t[ik][:, ck * 128 : (ck + 1) * 128],
                    rhs=x_t[:, ck, :nblk],
                    start=(ck == 0),
                    stop=(ck == CK - 1),
                )
            ga = gpool.tile([128, NBLK], f32, tag="ga")
            nc.scalar.activation(
                ga[:, :nblk], psg[:, :nblk], mybir.ActivationFunctionType.Silu
            )
            nc.vector.tensor_mul(h_t[:, ik, :nblk], ga[:, :nblk], psu[:, :nblk])

        for ck in range(CK):
            psd = pd.tile([128, NBLK], f32, tag="psd")
            for ik in range(IK):
                nc.tensor.matmul(
                    psd[:, :nblk],
                    lhsT=wd_t[ik][:, ck * 128 : (ck + 1) * 128],
                    rhs=h_t[:, ik, :nblk],
                    start=(ik == 0),
                    stop=(ik == IK - 1),
                )
            yb = ypool.tile([128, NBLK], f32, tag="yb")
            nc.vector.tensor_copy(yb[:, :nblk], psd[:, :nblk])
            nc.sync.dma_start(
                out=aps["y"][ck * 128 : (ck + 1) * 128, n0 : n0 + nblk],
                in_=yb[:, :nblk],
            )


def _build(cap_a, cap_b):
    """One launch: shared FFN (TPC tokens) + expert a (cap_a) + expert b."""
    from contextlib import ExitStack

    import concourse.tile as tile
    from concourse import bacc, mybir

    f32 = mybir.dt.float32
    bf16 = mybir.dt.bfloat16

    nc = bacc.Bacc("TRN2", target_bir_lowering=False, debug=False)
    phases = []
    for s, cap in (("s", TPC), ("a", cap_a), ("b", cap_b)):
        aps = {
            "x": nc.dram_tensor(f"x{s}", [C, cap], bf16, kind="ExternalInput").ap(),
            "wg": nc.dram_tensor(
                f"wg{s}", [IK * 128, CK * 128], bf16, kind="ExternalInput"
            ).ap(),
            "wu": nc.dram_tensor(
                f"wu{s}", [IK * 128, CK * 128], bf16, kind="ExternalInput"
            ).ap(),
            "wd": nc.dram_tensor(f"wd{s}", [I, C], bf16, kind="ExternalInput").ap(),
            "y": nc.dram_tensor(f"y{s}", [C, cap], f32, kind="ExternalOutput").ap(),
        }
        phases.append((aps, s, cap))

    with tile.TileContext(nc) as tc, ExitStack() as ctx:
        pools = {
            "w": ctx.enter_context(tc.tile_pool(name="wp", bufs=2)),
            "x": ctx.enter_context(tc.tile_pool(name="xp", bufs=3)),
            "h": ctx.enter_context(tc.tile_pool(name="hp", bufs=2)),
            "g": ctx.enter_context(tc.tile_pool(name="gp", bufs=2)),
            "y": ctx.enter_context(tc.tile_pool(name="yp", bufs=3)),
            "pgu": ctx.enter_context(tc.tile_pool(name="pgu", bufs=2, space="PSUM")),
            "pd": ctx.enter_context(tc.tile_pool(name="pd", bufs=2, space="PSUM")),
        }
        for aps, s, cap in phases:
            _emit_ffn_phase(nc, tc, pools, aps, s, cap)

    nc.compile()
    return nc


def _run(nc, in_maps, tag):
    from concourse.bass_utils import run_bass_kernel_spmd

    if TRACE:
        _enable_axon_ntff_profiling()
        res = run_bass_kernel_spmd(nc, in_maps, list(range(NCORE)), trace=True)
        LAST[f"{tag}_ns"] = res.exec_time_ns
        if res.instructions_and_trace is not None:
            LAST[f"{tag}_trace"] = res.instructions_and_trace[1]
    else:
        res = run_bass_kernel_spmd(nc, in_maps, list(range(NCORE)), trace=False)
    return res.results


def _ceil128(n):
    return max(128, ((n + 127) // 128) * 128)


def kernel(x, w_gate, expert_bias, wg, wu, wd, swg, swu, swd):
    LAST.clear()
    bf16 = _bf16()
    xf = np.ascontiguousarray(np.asarray(x, np.float32).reshape(NTOK, C))
    w_gate = np.asarray(w_gate, np.float32)
    expert_bias = np.asarray(expert_bias, np.float32)
    wg = np.asarray(wg, np.float32)
    wu = np.asarray(wu, np.float32)
    wd = np.asarray(wd, np.float32)

    # ---- host router: logits -> softmax -> top-2 -> renormalized weights
    logits = xf @ w_gate + expert_bias  # (N, E) f32
    m = logits.max(axis=1, keepdims=True)
    p = np.exp(logits - m, dtype=np.float32)
    p /= p.sum(axis=1, keepdims=True)
    top2 = np.argsort(-p, axis=1, kind="stable")[:, :TOPK]  # (N, 2)
    pv = np.take_along_axis(p, top2, axis=1)
    pv = pv / pv.sum(axis=1, keepdims=True)  # renormalized combine weights

    tok, wtok = [], []
    for e in range(E):
        sel0 = top2[:, 0] == e
        sel1 = top2[:, 1] == e
        ii = np.nonzero(sel0 | sel1)[0]
        ww = np.where(sel0, pv[:, 0], pv[:, 1])[ii].astype(np.float32)
        tok.append(ii)
        wtok.append(ww)
    counts = np.array([len(ii) for ii in tok])

    # ---- balanced pairing: core i gets (i-th largest, i-th smallest)
    order = np.argsort(-counts, kind="stable")
    slot_a = [int(order[i]) for i in range(NCORE)]
    slot_b = [int(order[E - 1 - i]) for i in range(NCORE)]
    cap_a = _ceil128(int(counts[slot_a[0]].max()))
    cap_b = _ceil128(int(max(counts[e] for e in slot_b)))

    key = (cap_a, cap_b)
    if key not in _progs:
        _progs[key] = _build(cap_a, cap_b)

    # ---- per-core inputs
    xf_bf = xf.astype(bf16)
    xt_bf = np.ascontiguousarray(xf_bf.T)  # (C, NTOK) bf16
    swg_p, swu_p = _pack_gu(swg), _pack_gu(swu)
    swd_bf = np.ascontiguousarray(swd.astype(bf16))

    in_maps = []
    for c in range(NCORE):
        m_ = {
            "xs": np.ascontiguousarray(xt_bf[:, c * TPC : (c + 1) * TPC]),
            "wgs": swg_p,
            "wus": swu_p,
            "wds": swd_bf,
        }
        for s, e, cap in (("a", slot_a[c], cap_a), ("b", slot_b[c], cap_b)):
            ii = tok[e]
            xt = np.zeros((C, cap), bf16)
            xt[:, : len(ii)] = xf_bf[ii].T
            m_[f"x{s}"] = xt
            m_[f"wg{s}"] = _pack_gu(wg[e])
            m_[f"wu{s}"] = _pack_gu(wu[e])
            m_[f"wd{s}"] = np.ascontiguousarray(wd[e].astype(bf16))
        in_maps.append(m_)

    res = _run(_progs[key], in_maps, "launch")

    # ---- host combine: shared + scatter-add of comb-weighted expert outputs
    out = np.empty((NTOK, C), np.float32)
    for c in range(NCORE):
        out[c * TPC : (c + 1) * TPC] = res[c]["ys"].T
    for s, slots in (("a", slot_a), ("b", slot_b)):
        for c, e in enumerate(slots):
            ii = tok[e]
            y = res[c][f"y{s}"][:, : len(ii)]  # (C, len), unscaled
            out[ii] += y.T * wtok[e][:, None]

    if TRACE:
        LAST["total_ns"] = sum(
            v for k, v in LAST.items() if isinstance(v, int) and k.endswith("_ns")
        )
    return out.reshape(B, T, C)


# revision 7
# speedup vs baseline: 1.3241x; 1.0020x over previous
"""MoE (16 routed experts, top-2, + shared expert) on 8 TRN2 NeuronCores.

Strategy (expert-parallel per the sharding hint):
  Host: router (x @ w_gate + bias, softmax, top-2, renormalize) — 0.1% of
    total FLOPs — plus the all-to-all dispatch: per-expert token gather into
    dense padded batches.  Experts are pair-balanced across cores (sorted
    pairing: core i gets the i-th largest and i-th smallest expert batch) so
    per-core padded work is near-uniform and minimal.
  Device (ONE SPMD launch, all 8 cores): core c runs the shared-expert
    SwiGLU FFN on its 2048-token slice, then the FFNs of its two routed
    experts on their gathered batches.  All matmul data travels bf16
    (PE runs bf16 at the same rate as fp32r, but DMA bytes halve); PSUM
    accumulation is fp32 and outputs are written fp32.
  Host: combine — scatter-add comb-weighted expert outputs + shared output.

Layout: all activations travel transposed (feature-major, token-minor).
Gate/up weights are host-packed per-ik ([IK*128, CK*128] with (ck, icol)
free order) so each 128-wide I-chunk loads as one contiguous DMA and the
first matmuls can start ~2us into the launch.  Weight DMAs ride SWDGE
(gpsimd) so their tile-recycle waits never block the x/y HWDGE queue.
"""

import numpy as np

# model dims (fixed for this problem)
E, TOPK, C, I = 16, 2, 768, 1536
B, T = 8, 2048
NCORE = 8
NTOK = B * T           # 16384
TPC = NTOK // NCORE    # 2048 tokens per core
CK = C // 128          # 6 contraction chunks for C
IK = I // 128          # 12 chunks for I
NBLK = 512             # token block = PE moving-dim per matmul

TRACE = False          # set True (from a driver) to capture NTFF timing
LAST = {}              # timing info from the most recent kernel() call

_progs = {}            # compiled program cache


def _bf16():
    import ml_dtypes

    return ml_dtypes.bfloat16


def _enable_axon_ntff_profiling():
    import sys
    import types

    if "antenv.axon_hooks" not in sys.modules:
        mod = types.ModuleType("antenv.axon_hooks")
        mod._hook = None
        mod.set_axon_ntff_profile_hook = lambda h: setattr(mod, "_hook", h)
        mod.get_axon_ntff_profile_hook = lambda: mod._hook
        sys.modules["antenv.axon_hooks"] = mod
    from antenv.axon_hooks import set_axon_ntff_profile_hook  # type: ignore
    from trn_agent_boot.trn_boot import _ntff_profile_via_ctypes

    set_axon_ntff_profile_hook(_ntff_profile_via_ctypes("/opt/axon/libaxon_pjrt.so"))
    import concourse.bass_utils as bu

    bu.upload_artifacts = lambda tmpdir: f"file://{tmpdir}"


def _blocks(m):
    out = []
    n0 = 0
    while n0 < m:
        nb = min(NBLK, m - n0)
        out.append((n0, nb))
        n0 += nb
    return out


def _pack_gu(wg, wu):
    """Gate+up [C, I] x2 -> [IK*128, 2*CK*128] bf16: row-block ik is one
    contiguous [128, 1536] DMA; free order (m, ck, icol) with m=gate|up."""
    s = np.stack([wg, wu])  # [2, C, I]
    p = s.reshape(2, CK, 128, IK, 128).transpose(3, 2, 0, 1, 4)
    p = p.reshape(IK * 128, 2 * CK * 128)
    return np.ascontiguousarray(p.astype(_bf16()))


def _pack_d(wd):
    """Down [I, C] -> [128, IK*C] bf16: one DMA; free order (ik, c)."""
    p = wd.reshape(IK, 128, C).transpose(1, 0, 2).reshape(128, IK * C)
    return np.ascontiguousarray(p.astype(_bf16()))


def _pack_x(xt):
    """[C, cap] bf16 -> [128, CK, cap]: one 3D DMA per token block."""
    return np.ascontiguousarray(xt.reshape(CK, 128, -1).transpose(1, 0, 2))


def _emit_ffn_phase(nc, tc, pools, aps, tag, cap):
    """Full SwiGLU FFN phase: y[C, cap] = down(silu(gate(x)) * up(x)).

    aps: dict with x (DRAM [C, cap] bf16), y (DRAM [C, cap] f32),
    wg/wu (DRAM [IK*128, CK*128] bf16 packed), wd (DRAM [I, C] bf16).
    """
    import concourse.mybir as mybir

    f32 = mybir.dt.float32
    bf16 = mybir.dt.bfloat16
    wpool, xpool, hpool, gpool, ypool, pgu, pd = (
        pools["w"],
        pools["x"],
        pools["h"],
        pools["g"],
        pools["y"],
        pools["pgu"],
        pools["pd"],
    )

    # weight tiles: per-ik fused gate+up [128, 2*CK*128]; down as one tile.
    # SWDGE so a blocked recycle-wait never stalls the x/y HWDGE queue.
    wgu_t = []
    for ik in range(IK):
        g = wpool.tile([128, 2 * CK * 128], bf16, tag=f"wgu{ik}")
        nc.gpsimd.dma_start(out=g[:], in_=aps["wgu"][ik * 128 : (ik + 1) * 128, :])
        wgu_t.append(g)
    wd_t = wpool.tile([128, IK * C], bf16, tag="wd")
    nc.gpsimd.dma_start(out=wd_t[:], in_=aps["wd"][:])

    for n0, nblk in _blocks(cap):
        x_t = xpool.tile([128, CK, NBLK], bf16, tag="x")
        nc.sync.dma_start(out=x_t[:, :, :nblk], in_=aps["x"][:, :, n0 : n0 + nblk])
        h_t = hpool.tile([128, IK, NBLK], bf16, tag="h")
        for ik in range(IK):
            psg = pgu.tile([128, NBLK], f32, tag="psg")
            psu = pgu.tile([128, NBLK], f32, tag="psu")
            for ck in range(CK):
                nc.tensor.matmul(
                    psg[:, :nblk],
                    lhsT=wgu_t[ik][:, ck * 128 : (ck + 1) * 128],
                    rhs=x_t[:, ck, :nblk],
                    start=(ck == 0),
                    stop=(ck == CK - 1),
                )
            for ck in range(CK):
                nc.tensor.matmul(
                    psu[:, :nblk],
                    lhsT=wgu_t[ik][:, CK * 128 + ck * 128 : CK * 128 + (ck + 1) * 128],
                    rhs=x_t[:, ck, :nblk],
                    start=(ck == 0),
                    stop=(ck == CK - 1),
                )
            ga = gpool.tile([128, NBLK], f32, tag="ga")
            nc.scalar.activation(
                ga[:, :nblk], psg[:, :nblk], mybir.ActivationFunctionType.Silu
            )
            nc.vector.tensor_mul(h_t[:, ik, :nblk], ga[:, :nblk], psu[:, :nblk])

        y_t = ypool.tile([128, CK, NBLK], f32, tag="y")
        for ck in range(CK):
            psd = pd.tile([128, NBLK], f32, tag="psd")
            for ik in range(IK):
                nc.tensor.matmul(
                    psd[:, :nblk],
                    lhsT=wd_t[:, ik * C + ck * 128 : ik * C + (ck + 1) * 128],
                    rhs=h_t[:, ik, :nblk],
                    start=(ik == 0),
                    stop=(ik == IK - 1),
                )
            nc.vector.tensor_copy(y_t[:, ck, :nblk], psd[:, :nblk])
        nc.sync.dma_start(out=aps["y"][:, :, n0 : n0 + nblk], in_=y_t[:, :, :nblk])


def _build(cap_a, cap_b):
    """One launch: shared FFN (TPC tokens) + expert a (cap_a) + expert b."""
    from contextlib import ExitStack

    import concourse.tile as tile
    from concourse import bacc, mybir

    f32 = mybir.dt.float32
    bf16 = mybir.dt.bfloat16

    nc = bacc.Bacc("TRN2", target_bir_lowering=False, debug=False)
    phases = []
    for s, cap in (("s", TPC), ("a", cap_a), ("b", cap_b)):
        aps = {
            "x": nc.dram_tensor(
                f"x{s}", [128, CK, cap], bf16, kind="ExternalInput"
            ).ap(),
            "wgu": nc.dram_tensor(
                f"wgu{s}", [IK * 128, 2 * CK * 128], bf16, kind="ExternalInput"
            ).ap(),
            "wd": nc.dram_tensor(
                f"wd{s}", [128, IK * C], bf16, kind="ExternalInput"
            ).ap(),
            "y": nc.dram_tensor(
                f"y{s}", [128, CK, cap], f32, kind="ExternalOutput"
            ).ap(),
        }
        phases.append((aps, s, cap))

    with tile.TileContext(nc) as tc, ExitStack() as ctx:
        pools = {
            "w": ctx.enter_context(tc.tile_pool(name="wp", bufs=2)),
            "x": ctx.enter_context(tc.tile_pool(name="xp", bufs=3)),
            "h": ctx.enter_context(tc.tile_pool(name="hp", bufs=2)),
            "g": ctx.enter_context(tc.tile_pool(name="gp", bufs=2)),
            "y": ctx.enter_context(tc.tile_pool(name="yp", bufs=2)),
            "pgu": ctx.enter_context(tc.tile_pool(name="pgu", bufs=2, space="PSUM")),
            "pd": ctx.enter_context(tc.tile_pool(name="pd", bufs=2, space="PSUM")),
        }
        for aps, s, cap in phases:
            _emit_ffn_phase(nc, tc, pools, aps, s, cap)

    nc.compile()
    return nc


def _run(nc, in_maps, tag):
    from concourse.bass_utils import run_bass_kernel_spmd

    if TRACE:
        _enable_axon_ntff_profiling()
        res = run_bass_kernel_spmd(nc, in_maps, list(range(NCORE)), trace=True)
        LAST[f"{tag}_ns"] = res.exec_time_ns
        if res.instructions_and_trace is not None:
            LAST[f"{tag}_trace"] = res.instructions_and_trace[1]
    else:
        res = run_bass_kernel_spmd(nc, in_maps, list(range(NCORE)), trace=False)
    return res.results


def _ceil128(n):
    return max(128, ((n + 127) // 128) * 128)


def kernel(x, w_gate, expert_bias, wg, wu, wd, swg, swu, swd):
    LAST.clear()
    bf16 = _bf16()
    xf = np.ascontiguousarray(np.asarray(x, np.float32).reshape(NTOK, C))
    w_gate = np.asarray(w_gate, np.float32)
    expert_bias = np.asarray(expert_bias, np.float32)
    wg = np.asarray(wg, np.float32)
    wu = np.asarray(wu, np.float32)
    wd = np.asarray(wd, np.float32)

    # ---- host router: logits -> softmax -> top-2 -> renormalized weights
    logits = xf @ w_gate + expert_bias  # (N, E) f32
    m = logits.max(axis=1, keepdims=True)
    p = np.exp(logits - m, dtype=np.float32)
    p /= p.sum(axis=1, keepdims=True)
    top2 = np.argsort(-p, axis=1, kind="stable")[:, :TOPK]  # (N, 2)
    pv = np.take_along_axis(p, top2, axis=1)
    pv = pv / pv.sum(axis=1, keepdims=True)  # renormalized combine weights

    tok, wtok = [], []
    for e in range(E):
        sel0 = top2[:, 0] == e
        sel1 = top2[:, 1] == e
        ii = np.nonzero(sel0 | sel1)[0]
        ww = np.where(sel0, pv[:, 0], pv[:, 1])[ii].astype(np.float32)
        tok.append(ii)
        wtok.append(ww)
    counts = np.array([len(ii) for ii in tok])

    # ---- balanced pairing: core i gets (i-th largest, i-th smallest)
    order = np.argsort(-counts, kind="stable")
    slot_a = [int(order[i]) for i in range(NCORE)]
    slot_b = [int(order[E - 1 - i]) for i in range(NCORE)]
    cap_a = _ceil128(int(counts[slot_a[0]].max()))
    cap_b = _ceil128(int(max(counts[e] for e in slot_b)))

    key = (cap_a, cap_b)
    if key not in _progs:
        _progs[key] = _build(cap_a, cap_b)

    # ---- per-core inputs
    xf_bf = xf.astype(bf16)
    xt_bf = np.ascontiguousarray(xf_bf.T)  # (C, NTOK) bf16
    swgu_p = _pack_gu(np.asarray(swg, np.float32), np.asarray(swu, np.float32))
    swd_p = _pack_d(np.asarray(swd, np.float32))

    in_maps = []
    for c in range(NCORE):
        m_ = {
            "xs": _pack_x(xt_bf[:, c * TPC : (c + 1) * TPC]),
            "wgus": swgu_p,
            "wds": swd_p,
        }
        for s, e, cap in (("a", slot_a[c], cap_a), ("b", slot_b[c], cap_b)):
            ii = tok[e]
            xt = np.zeros((C, cap), bf16)
            xt[:, : len(ii)] = xf_bf[ii].T
            m_[f"x{s}"] = _pack_x(xt)
            m_[f"wgu{s}"] = _pack_gu(wg[e], wu[e])
            m_[f"wd{s}"] = _pack_d(wd[e])
        in_maps.append(m_)

    res = _run(_progs[key], in_maps, "launch")

    # ---- host combine: shared + scatter-add of comb-weighted expert outputs
    def unpack_y(y3, cap):
        # [128, CK, cap] -> (cap, C)
        return y3.transpose(2, 1, 0).reshape(cap, C)

    out = np.empty((NTOK, C), np.float32)
    for c in range(NCORE):
        out[c * TPC : (c + 1) * TPC] = unpack_y(res[c]["ys"], TPC)
    for s, slots, cap in (("a", slot_a, cap_a), ("b", slot_b, cap_b)):
        for c, e in enumerate(slots):
            ii = tok[e]
            y = unpack_y(res[c][f"y{s}"], cap)[: len(ii)]  # (len, C), unscaled
            out[ii] += y * wtok[e][:, None]

    if TRACE:
        LAST["total_ns"] = sum(
            v for k, v in LAST.items() if isinstance(v, int) and k.endswith("_ns")
        )
    return out.reshape(B, T, C)


# revision 11
# speedup vs baseline: 1.3318x; 1.0058x over previous
"""MoE (16 routed experts, top-2, + shared expert) on 8 TRN2 NeuronCores.

Strategy (expert-parallel per the sharding hint):
  Host: router (x @ w_gate + bias, softmax, top-2, renormalize) — 0.1% of
    total FLOPs — plus the all-to-all dispatch: per-expert token gather into
    dense padded batches.  Experts are pair-balanced across cores (sorted
    pairing: core i gets the i-th largest and i-th smallest expert batch) so
    per-core padded work is near-uniform and minimal.
  Device (ONE SPMD launch, all 8 cores): core c runs the shared-expert
    SwiGLU FFN on its 2048-token slice, then the FFNs of its two routed
    experts on their gathered batches.  All matmul data travels bf16
    (PE runs bf16 at the same rate as fp32r, but DMA bytes halve); PSUM
    accumulation is fp32 and outputs are written fp32.
  Host: combine — scatter-add comb-weighted expert outputs + shared output.

Layout: all activations travel transposed (feature-major, token-minor).
Gate/up weights are host-packed per-ik ([IK*128, CK*128] with (ck, icol)
free order) so each 128-wide I-chunk loads as one contiguous DMA and the
first matmuls can start ~2us into the launch.  Weight DMAs ride SWDGE
(gpsimd) so their tile-recycle waits never block the x/y HWDGE queue.
"""

import numpy as np

# model dims (fixed for this problem)
E, TOPK, C, I = 16, 2, 768, 1536
B, T = 8, 2048
NCORE = 8
NTOK = B * T           # 16384
TPC = NTOK // NCORE    # 2048 tokens per core
CK = C // 128          # 6 contraction chunks for C
IK = I // 128          # 12 chunks for I
NBLK = 512             # token block = PE moving-dim per matmul

TRACE = False          # set True (from a driver) to capture NTFF timing
LAST = {}              # timing info from the most recent kernel() call

_progs = {}            # compiled program cache


def _bf16():
    import ml_dtypes

    return ml_dtypes.bfloat16


def _enable_axon_ntff_profiling():
    import sys
    import types

    if "antenv.axon_hooks" not in sys.modules:
        mod = types.ModuleType("antenv.axon_hooks")
        mod._hook = None
        mod.set_axon_ntff_profile_hook = lambda h: setattr(mod, "_hook", h)
        mod.get_axon_ntff_profile_hook = lambda: mod._hook
        sys.modules["antenv.axon_hooks"] = mod
    from antenv.axon_hooks import set_axon_ntff_profile_hook  # type: ignore
    from trn_agent_boot.trn_boot import _ntff_profile_via_ctypes

    set_axon_ntff_profile_hook(_ntff_profile_via_ctypes("/opt/axon/libaxon_pjrt.so"))
    import concourse.bass_utils as bu

    bu.upload_artifacts = lambda tmpdir: f"file://{tmpdir}"


def _blocks(m):
    out = []
    n0 = 0
    while n0 < m:
        nb = min(NBLK, m - n0)
        out.append((n0, nb))
        n0 += nb
    return out


def _pack_gu(wg, wu):
    """Gate+up [C, I] x2 -> [IK*128, 2*CK*128] bf16: row-block ik is one
    contiguous [128, 1536] DMA; free order (m, ck, icol) with m=gate|up."""
    s = np.stack([wg, wu])  # [2, C, I]
    p = s.reshape(2, CK, 128, IK, 128).transpose(3, 2, 0, 1, 4)
    p = p.reshape(IK * 128, 2 * CK * 128)
    return np.ascontiguousarray(p.astype(_bf16()))


def _pack_d(wd):
    """Down [I, C] -> [128, IK*C] bf16: one DMA; free order (ik, c)."""
    p = wd.reshape(IK, 128, C).transpose(1, 0, 2).reshape(128, IK * C)
    return np.ascontiguousarray(p.astype(_bf16()))


def _pack_x(xt):
    """[C, cap] bf16 -> [128, CK, cap]: one 3D DMA per token block."""
    return np.ascontiguousarray(xt.reshape(CK, 128, -1).transpose(1, 0, 2))


def _emit_ffn_phase(nc, tc, pools, aps, tag, cap):
    """Full SwiGLU FFN phase: y[C, cap] = down(silu(gate(x)) * up(x)).

    aps: dict with x (DRAM [C, cap] bf16), y (DRAM [C, cap] f32),
    wg/wu (DRAM [IK*128, CK*128] bf16 packed), wd (DRAM [I, C] bf16).
    """
    import concourse.mybir as mybir

    f32 = mybir.dt.float32
    bf16 = mybir.dt.bfloat16
    wpool, xpool, hpool, gpool, ypool, pgu, pd = (
        pools["w"],
        pools["x"],
        pools["h"],
        pools["g"],
        pools["y"],
        pools["pgu"],
        pools["pd"],
    )

    # weight tiles: per-ik fused gate+up [128, 2*CK*128]; down as one tile.
    # SWDGE so a blocked recycle-wait never stalls the x/y HWDGE queue.
    wgu_t = []
    for ik in range(IK):
        g = wpool.tile([128, 2 * CK * 128], bf16, tag=f"wgu{ik}")
        nc.gpsimd.dma_start(out=g[:], in_=aps["wgu"][ik * 128 : (ik + 1) * 128, :])
        wgu_t.append(g)
    wd_t = wpool.tile([128, IK * C], bf16, tag="wd")
    nc.gpsimd.dma_start(out=wd_t[:], in_=aps["wd"][:])

    for n0, nblk in _blocks(cap):
        x_t = xpool.tile([128, CK, NBLK], bf16, tag="x")
        nc.sync.dma_start(out=x_t[:, :, :nblk], in_=aps["x"][:, :, n0 : n0 + nblk])
        h_t = hpool.tile([128, IK, NBLK], bf16, tag="h")
        for ik in range(IK):
            psg = pgu.tile([128, NBLK], f32, tag="psg")
            psu = pgu.tile([128, NBLK], f32, tag="psu")
            for ck in range(CK):
                nc.tensor.matmul(
                    psg[:, :nblk],
                    lhsT=wgu_t[ik][:, ck * 128 : (ck + 1) * 128],
                    rhs=x_t[:, ck, :nblk],
                    start=(ck == 0),
                    stop=(ck == CK - 1),
                )
            for ck in range(CK):
                nc.tensor.matmul(
                    psu[:, :nblk],
                    lhsT=wgu_t[ik][:, CK * 128 + ck * 128 : CK * 128 + (ck + 1) * 128],
                    rhs=x_t[:, ck, :nblk],
                    start=(ck == 0),
                    stop=(ck == CK - 1),
                )
            ga = gpool.tile([128, NBLK], f32, tag="ga")
            nc.scalar.activation(
                ga[:, :nblk], psg[:, :nblk], mybir.ActivationFunctionType.Silu
            )
            nc.vector.tensor_mul(h_t[:, ik, :nblk], ga[:, :nblk], psu[:, :nblk])

        y_t = ypool.tile([128, CK, NBLK], f32, tag="y")
        for ck in range(CK):
            psd = pd.tile([128, NBLK], f32, tag="psd")
            for ik in range(IK):
                nc.tensor.matmul(
                    psd[:, :nblk],
                    lhsT=wd_t[:, ik * C + ck * 128 : ik * C + (ck + 1) * 128],
                    rhs=h_t[:, ik, :nblk],
                    start=(ik == 0),
                    stop=(ik == IK - 1),
                )
            nc.vector.tensor_copy(y_t[:, ck, :nblk], psd[:, :nblk])
            # per-ck store so the kernel tail only waits on the last chunk
            nc.sync.dma_start(
                out=aps["y"][:, ck, n0 : n0 + nblk], in_=y_t[:, ck, :nblk]
            )


def _build(cap_a, cap_b):
    """One launch: shared FFN (TPC tokens) + expert a (cap_a) + expert b."""
    from contextlib import ExitStack

    import concourse.tile as tile
    from concourse import bacc, mybir

    f32 = mybir.dt.float32
    bf16 = mybir.dt.bfloat16

    nc = bacc.Bacc("TRN2", target_bir_lowering=False, debug=False)
    phases = []
    for s, cap in (("s", TPC), ("a", cap_a), ("b", cap_b)):
        aps = {
            "x": nc.dram_tensor(
                f"x{s}", [128, CK, cap], bf16, kind="ExternalInput"
            ).ap(),
            "wgu": nc.dram_tensor(
                f"wgu{s}", [IK * 128, 2 * CK * 128], bf16, kind="ExternalInput"
            ).ap(),
            "wd": nc.dram_tensor(
                f"wd{s}", [128, IK * C], bf16, kind="ExternalInput"
            ).ap(),
            "y": nc.dram_tensor(
                f"y{s}", [128, CK, cap], f32, kind="ExternalOutput"
            ).ap(),
        }
        phases.append((aps, s, cap))

    with tile.TileContext(nc) as tc, ExitStack() as ctx:
        pools = {
            "w": ctx.enter_context(tc.tile_pool(name="wp", bufs=2)),
            "x": ctx.enter_context(tc.tile_pool(name="xp", bufs=3)),
            "h": ctx.enter_context(tc.tile_pool(name="hp", bufs=2)),
            "g": ctx.enter_context(tc.tile_pool(name="gp", bufs=2)),
            "y": ctx.enter_context(tc.tile_pool(name="yp", bufs=2)),
            "pgu": ctx.enter_context(tc.tile_pool(name="pgu", bufs=2, space="PSUM")),
            "pd": ctx.enter_context(tc.tile_pool(name="pd", bufs=2, space="PSUM")),
        }
        # HAM warmup: ~40 dep-free matmuls run during the framework preamble
        # and DMA ramp so the PE is already at 2.4 GHz when real work lands.
        with tc.tile_pool(name="pw", bufs=2, space="PSUM") as pw:
            warm = pools["g"].tile([128, 128], bf16, tag="warm")
            nc.vector.memset(warm[:], 0.0)
            for _ in range(40):
                pw_t = pw.tile([128, 128], f32, tag="w")
                nc.tensor.matmul(pw_t[:], lhsT=warm[:], rhs=warm[:], start=True, stop=True)
        for aps, s, cap in phases:
            _emit_ffn_phase(nc, tc, pools, aps, s, cap)

    nc.compile()
    return nc


def _run(nc, in_maps, tag):
    from concourse.bass_utils import run_bass_kernel_spmd

    if TRACE:
        _enable_axon_ntff_profiling()
        res = run_bass_kernel_spmd(nc, in_maps, list(range(NCORE)), trace=True)
        LAST[f"{tag}_ns"] = res.exec_time_ns
        if res.instructions_and_trace is not None:
            LAST[f"{tag}_trace"] = res.instructions_and_trace[1]
    else:
        res = run_bass_kernel_spmd(nc, in_maps, list(range(NCORE)), trace=False)
    return res.results


def _cap(n):
    # exact cap (any free-dim size works for matmul/DMA); floor for sanity
    return max(128, n)


def kernel(x, w_gate, expert_bias, wg, wu, wd, swg, swu, swd):
    LAST.clear()
    bf16 = _bf16()
    xf = np.ascontiguousarray(np.asarray(x, np.float32).reshape(NTOK, C))
    w_gate = np.asarray(w_gate, np.float32)
    expert_bias = np.asarray(expert_bias, np.float32)
    wg = np.asarray(wg, np.float32)
    wu = np.asarray(wu, np.float32)
    wd = np.asarray(wd, np.float32)

    # ---- host router: logits -> softmax -> top-2 -> renormalized weights
    logits = xf @ w_gate + expert_bias  # (N, E) f32
    m = logits.max(axis=1, keepdims=True)
    p = np.exp(logits - m, dtype=np.float32)
    p /= p.sum(axis=1, keepdims=True)
    top2 = np.argsort(-p, axis=1, kind="stable")[:, :TOPK]  # (N, 2)
    pv = np.take_along_axis(p, top2, axis=1)
    pv = pv / pv.sum(axis=1, keepdims=True)  # renormalized combine weights

    tok, wtok = [], []
    for e in range(E):
        sel0 = top2[:, 0] == e
        sel1 = top2[:, 1] == e
        ii = np.nonzero(sel0 | sel1)[0]
        ww = np.where(sel0, pv[:, 0], pv[:, 1])[ii].astype(np.float32)
        tok.append(ii)
        wtok.append(ww)
    counts = np.array([len(ii) for ii in tok])

    # ---- balanced pairing: core i gets (i-th largest, i-th smallest)
    order = np.argsort(-counts, kind="stable")
    slot_a = [int(order[i]) for i in range(NCORE)]
    slot_b = [int(order[E - 1 - i]) for i in range(NCORE)]
    cap_a = _cap(int(max(counts[e] for e in slot_a)))
    cap_b = _cap(int(max(counts[e] for e in slot_b)))

    key = (cap_a, cap_b)
    if key not in _progs:
        _progs[key] = _build(cap_a, cap_b)

    # ---- per-core inputs
    xf_bf = xf.astype(bf16)
    xt_bf = np.ascontiguousarray(xf_bf.T)  # (C, NTOK) bf16
    swgu_p = _pack_gu(np.asarray(swg, np.float32), np.asarray(swu, np.float32))
    swd_p = _pack_d(np.asarray(swd, np.float32))

    in_maps = []
    for c in range(NCORE):
        m_ = {
            "xs": _pack_x(xt_bf[:, c * TPC : (c + 1) * TPC]),
            "wgus": swgu_p,
            "wds": swd_p,
        }
        for s, e, cap in (("a", slot_a[c], cap_a), ("b", slot_b[c], cap_b)):
            ii = tok[e]
            xt = np.zeros((C, cap), bf16)
            xt[:, : len(ii)] = xf_bf[ii].T
            m_[f"x{s}"] = _pack_x(xt)
            m_[f"wgu{s}"] = _pack_gu(wg[e], wu[e])
            m_[f"wd{s}"] = _pack_d(wd[e])
        in_maps.append(m_)

    res = _run(_progs[key], in_maps, "launch")

    # ---- host combine: shared + scatter-add of comb-weighted expert outputs
    def unpack_y(y3, cap):
        # [128, CK, cap] -> (cap, C)
        return y3.transpose(2, 1, 0).reshape(cap, C)

    out = np.empty((NTOK, C), np.float32)
    for c in range(NCORE):
        out[c * TPC : (c + 1) * TPC] = unpack_y(res[c]["ys"], TPC)
    for s, slots, cap in (("a", slot_a, cap_a), ("b", slot_b, cap_b)):
        for c, e in enumerate(slots):
            ii = tok[e]
            y = unpack_y(res[c][f"y{s}"], cap)[: len(ii)]  # (len, C), unscaled
            out[ii] += y * wtok[e][:, None]

    if TRACE:
        LAST["total_ns"] = sum(
            v for k, v in LAST.items() if isinstance(v, int) and k.endswith("_ns")
        )
    return out.reshape(B, T, C)
